# revision 36
# baseline (speedup 1.0000x reference)
"""Multi-head attention on 8 TRN2 NeuronCores (Bass/Tile, SPMD).

Sharding: tensor-parallel over heads (2 heads/core) for qkv + attention,
then per-batch AllToAll rounds to token-sharded layout for the output
projection (each core projects 4 x 256-token slices).

Per-core layouts:
  xT      [NCH, 128, 4096] f16 -- x transposed+chunked (host-prepped):
                                  chunk ch, partition p, (d-group, 512 tok)
  wqkv    [128, 8, 384]    f16 -- [qA qB | kA kB | vA vB] cols, d-group-major
  bqkv    [128, 3]         f32 -- per-partition bias, col j = q/k/v
  wproj   [128, 8, 1024]   f16 -- rows grouped by source core
  bproj   [128, 8]         f32 -- col j = bias for out-dim tile j
  out     [D, TS]          f16 -- projected output, transposed; col block
                                  b*256.. holds round-b tokens

Inside: q^T,k^T [128(2 heads x 64dh), TOK] f16 in SBUF; v transposed back
to natural [k, dh] layout (PE transpose) with a ones column appended so
the PV matmul also produces softmax denominators (row 64).  Scores are
computed transposed (S^T[k,q]); the two heads' score matmuls are packed
as concurrent row-tiles (K=64 each: partitions 0-63 / 64-127).  exp()
without max-subtraction but with a constant -log(16) bias to keep fp16
sums in range (cancels in normalization).  Normalization is deferred
past the AllToAll: unnormalized pv + denominators ship f16; each round
normalizes with one ln+exp per 16 rows.  QKV/proj matmuls are
interleaved into attention emission to fill PE gaps behind the exp
chain (engine queues are FIFO).
"""

import numpy as np

import concourse.bass as bass
import concourse.mybir as mybir
import concourse.tile as tile
from concourse import bacc
from concourse.bass_utils import run_bass_kernel_spmd
from concourse.masks import make_identity

# Route `exp` activations to the natural_log_exp_and_others table set so
# exp and ln share one ACT table load (otherwise the table pass thrashes
# 2.7us loads between exp_and_others and the ln set on every normalize).
import concourse.bacc as _bacc_mod
from concourse.hw_specs import get_activation_tables as _orig_gat


def _gat_exp_with_ln(arch):
    d = dict(_orig_gat(arch))
    for name in d:
        if "exp" in name and "natural_log" not in name:
            d[name] = d[name] - {mybir.ActivationFunctionType.Exp}
    return d


_bacc_mod.get_activation_tables = _gat_exp_with_ln

# problem dims (fixed by the harness contract)
B, T, D, H = 4, 2048, 1024, 16
DH = D // H          # 64
NCORES = 8
HPC = H // NCORES    # 2 heads per core

F32 = mybir.dt.float32
FP16 = mybir.dt.float16
EXP = mybir.ActivationFunctionType.Exp
LOG = mybir.ActivationFunctionType.Ln
EXP_BIAS = -2.772588722239781  # -ln(16): keeps f16 pv/den in range


def emit(tc, io):
    nc = tc.nc
    TOK = B * T
    NCH = TOK // 512          # 16 x 512-token chunks
    CPB = T // 512            # 4 chunks per batch
    NQC = T // 512            # q-chunks per batch
    NKT = T // 128            # k-tiles per batch
    TS = TOK // NCORES        # 1024 output tokens per core
    RS = 512                  # max tokens per core per round

    xT, wqkv, bqkv, wproj, bproj, out = (
        io["xT"], io["wqkv"], io["bqkv"], io["wproj"], io["bproj"], io["out"])

    with tc.tile_pool(name="consts", bufs=1) as consts, \
         tc.tile_pool(name="bigs", bufs=1) as bigs, \
         tc.tile_pool(name="dram", bufs=1, space="DRAM") as dram:
        # ---- constants (single contiguous DMAs, all on the scalar queue so
        # the sync queue starts with the first xt chunk) ----
        w_sb = consts.tile([128, 8, 384], FP16)
        nc.scalar.dma_start(out=w_sb, in_=wqkv)
        bias_sb = consts.tile([128, 3], F32)
        nc.scalar.dma_start(out=bias_sb, in_=bqkv)
        bproj_sb = consts.tile([128, 8], F32)
        nc.scalar.dma_start(out=bproj_sb, in_=bproj)
        wproj_sb = consts.tile([128, 8, 1024], FP16)
        nc.scalar.dma_start(out=wproj_sb, in_=wproj)
        ident = consts.tile([128, 128], F32)
        make_identity(nc, ident)
        expb = consts.tile([128, 1], F32)
        nc.vector.memset(expb, EXP_BIAS)
        # band masks [j][pk, h, fq] = 1 if pk + 128*j <= fq else 0 (both heads)
        masks = consts.tile([128, 4, 2, 512], FP16)
        nc.vector.memset(masks, 1.0)
        for j in range(4):
            nc.gpsimd.affine_select(
                out=masks[:, j], in_=masks[:, j],
                compare_op=mybir.AluOpType.is_ge, fill=0.0,
                base=-128 * j, pattern=[[0, 2], [1, 512]],
                channel_multiplier=-1)

        # ---- big persistent buffers ----
        qT_sb = bigs.tile([128, TOK], FP16)
        kT_sb = bigs.tile([128, TOK], FP16)
        vn_sb = bigs.tile([128, B, HPC, NKT, 65], FP16)
        nc.vector.memset(vn_sb[:, :, :, :, 64:65], 1.0)
        # softmax denominators: row 0 = head0, row 64 = head1; cols (b, qc, x).
        # Reciprocals are computed here BEFORE the a2a (local deps only) so no
        # collective-gated op ever sits in the ACT queue blocking exps.
        den_sb = bigs.tile([65, B, 2048], FP16)
        nc.vector.memset(den_sb, 1.0)      # rows 1-63 stay 1 -> ln/exp benign
        lg_sb = bigs.tile([65, 2048], FP16)

        # per-round AllToAll buffers: [dst core, 128 attn rows + 2 denom, RS_r]
        # rounds 0-2 = batches 0-2 (256 tok/core); rounds 3,4 = batch-3
        # halves (128 tok/core) so the last collective+proj tail is short
        RSL = [256, 256, 256, 128, 128]
        COL0 = [0, 256, 512, 768, 896]
        NR = len(RSL)
        a2a_in = [dram.tile([NCORES, 130, RSL[r]], FP16, name=f"a2ain{r}")
                  for r in range(NR)]
        a2a_out = [dram.tile([NCORES, 130, RSL[r]], FP16, name=f"a2aout{r}")
                   for r in range(NR)]

        with tc.tile_pool(name="xt", bufs=4) as xt_pool, \
             tc.tile_pool(name="vstage", bufs=2) as vstage_pool, \
             tc.tile_pool(name="psall", bufs=1, space="PSUM") as ps_all, \
             tc.tile_pool(name="expp", bufs=4) as exp_pool, \
             tc.tile_pool(name="attp", bufs=3) as att_pool, \
             tc.tile_pool(name="projp", bufs=2) as proj_pool, \
             tc.tile_pool(name="otp", bufs=3) as out_pool:

            # ---------------- qkv (emitted as steps for interleaving) -------
            def qkv_steps(b):
                """Yield once per ct-group: 8 matmuls + bias (+v transpose)."""
                for ci in range(CPB):
                    ch = b * CPB + ci
                    tt0 = ci * 4
                    xt = xt_pool.tile([128, 8, 512], FP16, tag="xt",
                                      name=f"xt{ch}")
                    nc.sync.dma_start(
                        out=xt,
                        in_=xT[ch].rearrange("p (d t) -> p d t", d=8))
                    for ct in range(3):
                        ps = ps_all.tile([128, 512], F32, tag="qkvps", bufs=2,
                                         name=f"qkvps{ch}_{ct}")
                        for d in range(4):
                            nc.tensor.matmul(
                                ps,
                                w_sb[:, d, ct * 128:(ct + 1) * 128],
                                xt[:, d, :],
                                start=(d == 0), stop=False)
                        yield              # half-step: finer filler granularity
                        for d in range(4, 8):
                            nc.tensor.matmul(
                                ps,
                                w_sb[:, d, ct * 128:(ct + 1) * 128],
                                xt[:, d, :],
                                start=False, stop=(d == 7))
                        if ct == 0:
                            nc.vector.tensor_scalar_add(
                                qT_sb[:, ch * 512:(ch + 1) * 512], ps,
                                bias_sb[:, 0:1])
                        elif ct == 1:
                            nc.vector.tensor_scalar_add(
                                kT_sb[:, ch * 512:(ch + 1) * 512], ps,
                                bias_sb[:, 1:2])
                        else:
                            vst = vstage_pool.tile([128, 512], F32, tag="vst",
                                                   name=f"vst{ch}")
                            nc.vector.tensor_scalar_add(vst, ps, bias_sb[:, 2:3])
                            vtp = ps_all.tile([128, 512], F32, tag="qkvps",
                                              bufs=2, name=f"vtps{ch}")
                            for sub in range(4):
                                nc.tensor.transpose(
                                    vtp[:, sub * 128:(sub + 1) * 128],
                                    vst[:, sub * 128:(sub + 1) * 128],
                                    ident)
                            for hh in range(HPC):
                                nc.vector.tensor_copy(
                                    vn_sb[:, b, hh, tt0:tt0 + 4, 0:64],
                                    vtp.rearrange("p (s x) -> p s x", s=4)[
                                        :, :, hh * 64:(hh + 1) * 64])
                        yield

            # ---------------- projection round (emitted as steps) -----------
            def proj_steps(r):
                W = RSL[r]
                c0 = COL0[r]
                # step 0: fetch + normalize
                rhs = proj_pool.tile([128, 8, RS], FP16, tag="rhs",
                                     name=f"rhs{r}")
                nc.gpsimd.dma_start(
                    out=rhs[:, :, 0:W], in_=a2a_out[r][:, 0:128, :].rearrange(
                        "s p x -> p s x"))
                # broadcast the shipped reciprocal rows straight from a2a_out
                rcb = proj_pool.tile([128, 8, RS], FP16, tag="rcb",
                                     name=f"rcb{r}")
                for hh in range(HPC):
                    nc.gpsimd.dma_start(
                        out=rcb[hh * 64:(hh + 1) * 64, :, 0:W],
                        in_=bass.AP(tensor=a2a_out[r].tensor,
                                    offset=(128 + hh) * W,
                                    ap=[[0, 64], [130 * W, 8], [1, W]]))
                # normalize on GPSIMD: its queue is ordered behind the
                # collective anyway, so no other engine's FIFO can head-block
                rhn = proj_pool.tile([128, 8, RS], FP16, tag="rhn",
                                     name=f"rhn{r}")
                nc.gpsimd.tensor_mul(rhn[:, :, 0:W], rhs[:, :, 0:W],
                                     rcb[:, :, 0:W])
                yield
                for od in range(8):
                    pj = ps_all.tile([128, 512], F32, tag="qkvps", bufs=2,
                                     name=f"pj{r}_{od}")
                    for i in range(8):
                        nc.tensor.matmul(
                            pj[:, 0:W],
                            wproj_sb[:, i, od * 128:(od + 1) * 128],
                            rhn[:, i, 0:W],
                            start=(i == 0), stop=(i == 7))
                    ot = out_pool.tile([128, RS], FP16, tag="ot",
                                       name=f"ot{r}_{od}")
                    nc.vector.tensor_scalar_add(ot[:, 0:W], pj[:, 0:W],
                                                bproj_sb[:, od:od + 1])
                    nc.sync.dma_start(
                        out=out[od * 128:(od + 1) * 128, c0:c0 + W],
                        in_=ot[:, 0:W])
                    yield

            # ---------------- attention with filler interleave -------------
            def pump(filler):
                """Emit one filler step; return False when empty."""
                while filler:
                    try:
                        next(filler[0])
                        return True
                    except StopIteration:
                        filler.pop(0)
                return False

            def attention_batch(b, filler, qc_min=None, after_qc=None):
                pumped = 0
                for qc in range(NQC):
                    if qc_min is not None:
                        while pumped < qc_min[qc] and pump(filler):
                            pumped += 1
                    nkt_q = 4 * qc + 4
                    t0 = b * T
                    q0 = qT_sb[0:64, t0 + qc * 512: t0 + (qc + 1) * 512]
                    q1 = qT_sb[64:128, t0 + qc * 512: t0 + (qc + 1) * 512]
                    pv0 = ps_all.tile([65, 512], F32, tag="pv", bufs=2,
                                      name=f"pv0_{b}_{qc}")
                    pv1 = ps_all.tile([65, 512], F32, tag="pv", bufs=2,
                                      name=f"pv1_{b}_{qc}")
                    for kt in range(nkt_q):
                        sc = ps_all.tile([128, 1024], F32, tag="sc", bufs=2,
                                         name=f"sc{b}_{qc}_{kt}")
                        nc.tensor.matmul(
                            sc[:, 0:512],
                            kT_sb[0:64, t0 + kt * 128: t0 + (kt + 1) * 128],
                            q0, start=True, stop=True)
                        nc.tensor.matmul(
                            sc[:, 512:1024],
                            kT_sb[64:128, t0 + kt * 128: t0 + (kt + 1) * 128],
                            q1, start=True, stop=True)
                        ex = exp_pool.tile([128, 1024], FP16, tag="ex",
                                           name=f"ex{b}_{qc}_{kt}")
                        nc.scalar.activation(ex, sc, EXP, scale=0.125,
                                             bias=expb)
                        bj = kt - 4 * qc
                        if bj >= 0:
                            exr = ex.rearrange("p (h x) -> p h x", h=2)
                            nc.vector.tensor_mul(exr, exr, masks[:, bj])
                        # fill the PE gap behind the exp chain
                        if pump(filler):
                            pumped += 1
                        nc.tensor.matmul(
                            pv0, vn_sb[:, b, 0, kt, :], ex[:, 0:512],
                            start=(kt == 0), stop=(kt == nkt_q - 1))
                        nc.tensor.matmul(
                            pv1, vn_sb[:, b, 1, kt, :], ex[:, 512:1024],
                            start=(kt == 0), stop=(kt == nkt_q - 1))
                    # stage unnormalized output + denominators, ship to a2a
                    at = att_pool.tile([128, 512], FP16, tag="at",
                                       name=f"at{b}_{qc}")
                    nc.vector.tensor_copy(at[0:64, :], pv0[0:64, :])
                    nc.vector.tensor_copy(at[64:128, :], pv1[0:64, :])
                    nc.vector.tensor_copy(
                        den_sb[0:1, b, qc * 512:(qc + 1) * 512], pv0[64:65, :])
                    nc.vector.tensor_copy(
                        den_sb[64:65, b, qc * 512:(qc + 1) * 512], pv1[64:65, :])
                    r, d0, nd = rdst(b, qc)
                    nc.sync.dma_start(
                        out=a2a_in[r][d0:d0 + nd, 0:128, :].rearrange(
                            "d p x -> p d x"),
                        in_=at.rearrange("p (d x) -> p d x", d=nd))
                    if after_qc and qc in after_qc:
                        after_qc[qc]()

            def rdst(b, qc):
                if b < 3:
                    return b, 2 * qc, 2
                return 3 + qc // 2, 4 * (qc % 2), 4

            def rc_chain(b, q0, q1):
                """1/denominator for qc in [q0,q1) + ship rc rows to a2a_in."""
                sl = slice(q0 * 512, q1 * 512)
                nc.scalar.activation(lg_sb[:, sl], den_sb[:, b, sl], LOG)
                nc.scalar.activation(den_sb[:, b, sl], lg_sb[:, sl], EXP,
                                     scale=-1.0)
                for qc in range(q0, q1):
                    r, d0, nd = rdst(b, qc)
                    for hh, prow in ((0, 0), (1, 64)):
                        nc.sync.dma_start(
                            out=a2a_in[r][d0:d0 + nd, 128 + hh:129 + hh,
                                          :].rearrange("d h x -> h d x"),
                            in_=den_sb[prow:prow + 1, b,
                                       qc * 512:(qc + 1) * 512].rearrange(
                                           "p (d x) -> p d x", d=nd))

            def a2a(r):
                nc.gpsimd.collective_compute(
                    "AllToAll", mybir.AluOpType.bypass,
                    replica_groups=[list(range(NCORES))],
                    ins=[a2a_in[r][:]], outs=[a2a_out[r][:]])

            def flush(filler):
                while filler:
                    try:
                        next(filler[0])
                    except StopIteration:
                        filler.pop(0)

            # ---------------- schedule ----------------
            # dummy collective: absorb the one-time CC rendezvous/warmup cost
            # during the qkv prologue instead of in front of a2a(0)
            wz = consts.tile([8, 64], FP16)
            nc.vector.memset(wz, 0.0)
            warm_in = dram.tile([NCORES, 1, 8], FP16, name="warmin")
            warm_out = dram.tile([NCORES, 1, 8], FP16, name="warmout")
            nc.gpsimd.dma_start(out=warm_in, in_=wz[:, 0:8])
            nc.gpsimd.collective_compute(
                "AllToAll", mybir.AluOpType.bypass,
                replica_groups=[list(range(NCORES))],
                ins=[warm_in[:]], outs=[warm_out[:]])

            g0 = qkv_steps(0)
            for _ in range(6):             # chunk 0 of batch 0
                next(g0)
            f = [g0, qkv_steps(1)]
            attention_batch(0, f, qc_min=[0, 6, 12, 18])
            flush(f)                       # qkv(1) complete before att(1)
            rc_chain(0, 0, 4)
            a2a(0)
            f = [qkv_steps(2)]
            attention_batch(1, f)
            flush(f)
            rc_chain(1, 0, 4)
            a2a(1)
            f = [qkv_steps(3), proj_steps(0)]
            attention_batch(2, f)
            flush(f)                       # qkv(3) complete before att(3)
            rc_chain(2, 0, 4)
            a2a(2)

            def mid3():
                rc_chain(3, 0, 2)
                a2a(3)
                f.append(proj_steps(3))

            f = [proj_steps(1), proj_steps(2)]
            attention_batch(3, f, after_qc={1: mid3})
            rc_chain(3, 2, 4)
            a2a(4)
            f.append(proj_steps(4))
            flush(f)


def build_nc():
    TOK = B * T
    NCH = TOK // 512
    TS = TOK // NCORES
    nc = bacc.Bacc("TRN2", target_bir_lowering=False, debug=False,
                   enable_asserts=False, num_devices=NCORES)
    io = {
        "xT": nc.dram_tensor("xT", [NCH, 128, 4096], FP16,
                             kind="ExternalInput").ap(),
        "wqkv": nc.dram_tensor("wqkv", [128, 8, 384], FP16,
                               kind="ExternalInput").ap(),
        "bqkv": nc.dram_tensor("bqkv", [128, 3], F32,
                               kind="ExternalInput").ap(),
        "wproj": nc.dram_tensor("wproj", [128, 8, 1024], FP16,
                                kind="ExternalInput").ap(),
        "bproj": nc.dram_tensor("bproj", [128, 8], F32,
                                kind="ExternalInput").ap(),
        "out": nc.dram_tensor("out", [D, TS], FP16,
                              kind="ExternalOutput").ap(),
    }
    with tile.TileContext(nc) as tc:
        emit(tc, io)
    nc.compile()
    return nc


def make_in_maps(x, W_qkv, b_qkv, W_proj, b_proj):
    """Shard host inputs per core."""
    TOK = B * T
    NCH = TOK // 512
    x2 = np.asarray(x, np.float32).reshape(TOK, D)
    # [NCH, 128, 8, 512]: chunk, partition, d-group, token -- contiguous DMA
    xc = np.ascontiguousarray(
        x2.reshape(NCH, 512, 8, 128).transpose(0, 3, 2, 1)
    ).astype(np.float16).reshape(NCH, 128, 4096)
    wproj_f16 = np.ascontiguousarray(
        np.asarray(W_proj, np.float32).reshape(8, 128, D).transpose(1, 0, 2)
    ).astype(np.float16)
    bproj_rs = np.ascontiguousarray(
        np.asarray(b_proj, np.float32).reshape(8, 128).T)  # [128, 8]
    in_maps = []
    for c in range(NCORES):
        cols = []
        bcols = []
        for part in range(3):                            # q, k, v
            for h in (2 * c, 2 * c + 1):
                sl = slice(part * D + h * DH, part * D + (h + 1) * DH)
                cols.append(np.asarray(W_qkv, np.float32)[:, sl])
                bcols.append(np.asarray(b_qkv, np.float32)[sl])
        wq = np.concatenate(cols, axis=1)                # [1024, 384]
        wq = np.ascontiguousarray(
            wq.reshape(8, 128, 384).transpose(1, 0, 2)).astype(np.float16)
        bq = np.ascontiguousarray(
            np.concatenate(bcols).reshape(3, 128).T)     # [128, 3]
        in_maps.append({
            "xT": xc, "wqkv": wq, "bqkv": bq,
            "wproj": wproj_f16, "bproj": bproj_rs,
        })
    return in_maps


def gather_out(results):
    TOK = B * T
    fullT = np.empty((D, TOK), np.float32)
    for c in range(NCORES):
        o = results[c]["out"].astype(np.float32)
        for r in range(3):
            tok0 = r * T + (c // 2) * 512 + (c % 2) * 256
            fullT[:, tok0:tok0 + 256] = o[:, r * 256:(r + 1) * 256]
        for r in (3, 4):
            qc = 2 * (r - 3) + c // 4
            tok0 = 3 * T + qc * 512 + (c % 4) * 128
            col0 = 768 + (r - 3) * 128
            fullT[:, tok0:tok0 + 128] = o[:, col0:col0 + 128]
    return np.ascontiguousarray(fullT.T).reshape(B, T, D)


_NC_CACHE = {}
LAST_EXEC_NS = None


def kernel(x, mask, W_qkv, b_qkv, W_proj, b_proj, trace=False):
    global LAST_EXEC_NS
    key = (B, T)
    if key not in _NC_CACHE:
        _NC_CACHE[key] = build_nc()
    nc = _NC_CACHE[key]
    in_maps = make_in_maps(x, W_qkv, b_qkv, W_proj, b_proj)
    res = run_bass_kernel_spmd(nc, in_maps, core_ids=list(range(NCORES)),
                               trace=trace)
    LAST_EXEC_NS = res.exec_time_ns
    return gather_out(res.results)


# revision 37
# speedup vs baseline: 1.0117x; 1.0117x over previous
"""Multi-head attention on 8 TRN2 NeuronCores (Bass/Tile, SPMD).

Sharding: tensor-parallel over heads (2 heads/core) for qkv + attention,
then per-batch AllToAll rounds to token-sharded layout for the output
projection (each core projects 4 x 256-token slices).

Per-core layouts:
  xT      [NCH, 128, 4096] f16 -- x transposed+chunked (host-prepped):
                                  chunk ch, partition p, (d-group, 512 tok)
  wqkv    [128, 8, 384]    f16 -- [qA qB | kA kB | vA vB] cols, d-group-major
  bqkv    [128, 3]         f32 -- per-partition bias, col j = q/k/v
  wproj   [128, 8, 1024]   f16 -- rows grouped by source core
  bproj   [128, 8]         f32 -- col j = bias for out-dim tile j
  out     [D, TS]          f16 -- projected output, transposed; col block
                                  b*256.. holds round-b tokens

Inside: q^T,k^T [128(2 heads x 64dh), TOK] f16 in SBUF; v transposed back
to natural [k, dh] layout (PE transpose) with a ones column appended so
the PV matmul also produces softmax denominators (row 64).  Scores are
computed transposed (S^T[k,q]); the two heads' score matmuls are packed
as concurrent row-tiles (K=64 each: partitions 0-63 / 64-127).  exp()
without max-subtraction but with a constant -log(16) bias to keep fp16
sums in range (cancels in normalization).  Normalization is deferred
past the AllToAll: unnormalized pv + denominators ship f16; each round
normalizes with one ln+exp per 16 rows.  QKV/proj matmuls are
interleaved into attention emission to fill PE gaps behind the exp
chain (engine queues are FIFO).
"""

import numpy as np

import concourse.bass as bass
import concourse.mybir as mybir
import concourse.tile as tile
from concourse import bacc
from concourse.bass_utils import run_bass_kernel_spmd
from concourse.masks import make_identity

# Route `exp` activations to the natural_log_exp_and_others table set so
# exp and ln share one ACT table load (otherwise the table pass thrashes
# 2.7us loads between exp_and_others and the ln set on every normalize).
import concourse.bacc as _bacc_mod
from concourse.hw_specs import get_activation_tables as _orig_gat


def _gat_exp_with_ln(arch):
    d = dict(_orig_gat(arch))
    for name in d:
        if "exp" in name and "natural_log" not in name:
            d[name] = d[name] - {mybir.ActivationFunctionType.Exp}
    return d


_bacc_mod.get_activation_tables = _gat_exp_with_ln

# problem dims (fixed by the harness contract)
B, T, D, H = 4, 2048, 1024, 16
DH = D // H          # 64
NCORES = 8
HPC = H // NCORES    # 2 heads per core

F32 = mybir.dt.float32
FP16 = mybir.dt.float16
EXP = mybir.ActivationFunctionType.Exp
LOG = mybir.ActivationFunctionType.Ln
EXP_BIAS = -2.772588722239781  # -ln(16): keeps f16 pv/den in range


def emit(tc, io):
    nc = tc.nc
    TOK = B * T
    NCH = TOK // 512          # 16 x 512-token chunks
    CPB = T // 512            # 4 chunks per batch
    NQC = T // 512            # q-chunks per batch
    NKT = T // 128            # k-tiles per batch
    TS = TOK // NCORES        # 1024 output tokens per core
    RS = 512                  # max tokens per core per round

    xT, wqkv, bqkv, wproj, bproj, out = (
        io["xT"], io["wqkv"], io["bqkv"], io["wproj"], io["bproj"], io["out"])

    with tc.tile_pool(name="consts", bufs=1) as consts, \
         tc.tile_pool(name="bigs", bufs=1) as bigs, \
         tc.tile_pool(name="dram", bufs=1, space="DRAM") as dram:
        # ---- constants (single contiguous DMAs, all on the scalar queue so
        # the sync queue starts with the first xt chunk) ----
        w_sb = consts.tile([128, 8, 384], FP16)
        nc.scalar.dma_start(out=w_sb, in_=wqkv)
        bias_sb = consts.tile([128, 3], F32)
        nc.scalar.dma_start(out=bias_sb, in_=bqkv)
        bproj_sb = consts.tile([128, 8], F32)
        nc.scalar.dma_start(out=bproj_sb, in_=bproj)
        wproj_sb = consts.tile([128, 8, 1024], FP16)
        nc.scalar.dma_start(out=wproj_sb, in_=wproj)
        ident = consts.tile([128, 128], F32)
        make_identity(nc, ident)
        expb = consts.tile([128, 1], F32)
        nc.vector.memset(expb, EXP_BIAS)
        # band masks [j][pk, h, fq] = 1 if pk + 128*j <= fq else 0 (both heads)
        masks = consts.tile([128, 4, 2, 512], FP16)
        nc.vector.memset(masks, 1.0)
        for j in range(4):
            nc.gpsimd.affine_select(
                out=masks[:, j], in_=masks[:, j],
                compare_op=mybir.AluOpType.is_ge, fill=0.0,
                base=-128 * j, pattern=[[0, 2], [1, 512]],
                channel_multiplier=-1)

        # ---- big persistent buffers ----
        qT_sb = bigs.tile([128, TOK], FP16)
        kT_sb = bigs.tile([128, TOK], FP16)
        vn_sb = bigs.tile([128, B, HPC, NKT, 65], FP16)
        nc.vector.memset(vn_sb[:, :, :, :, 64:65], 1.0)
        # softmax denominators: row 0 = head0, row 64 = head1; cols (b, qc, x).
        # Reciprocals are computed here BEFORE the a2a (local deps only) so no
        # collective-gated op ever sits in the ACT queue blocking exps.
        den_sb = bigs.tile([65, B, 2048], FP16)
        nc.vector.memset(den_sb, 1.0)      # rows 1-63 stay 1 -> ln/exp benign
        lg_sb = bigs.tile([65, 2048], FP16)

        # per-round AllToAll buffers: [dst core, 128 attn rows + 2 denom, RS_r]
        # rounds 0-2 = batches 0-2 (256 tok/core); rounds 3,4 = batch-3
        # halves (128 tok/core) so the last collective+proj tail is short
        RSL = [256, 256, 256, 128, 128]
        COL0 = [0, 256, 512, 768, 896]
        NR = len(RSL)
        a2a_in = [dram.tile([NCORES, 130, RSL[r]], FP16, name=f"a2ain{r}")
                  for r in range(NR)]
        a2a_out = [dram.tile([NCORES, 130, RSL[r]], FP16, name=f"a2aout{r}")
                   for r in range(NR)]

        with tc.tile_pool(name="xt", bufs=3) as xt_pool, \
             tc.tile_pool(name="vstage", bufs=2) as vstage_pool, \
             tc.tile_pool(name="psall", bufs=1, space="PSUM") as ps_all, \
             tc.tile_pool(name="expp", bufs=3) as exp_pool, \
             tc.tile_pool(name="attp", bufs=2) as att_pool, \
             tc.tile_pool(name="projp", bufs=2) as proj_pool, \
             tc.tile_pool(name="otp", bufs=3) as out_pool:

            # ---------------- qkv (emitted as steps for interleaving) -------
            def qkv_steps(b):
                """Yield once per ct-group: 8 matmuls + bias (+v transpose)."""
                for ci in range(CPB):
                    ch = b * CPB + ci
                    tt0 = ci * 4
                    xt = xt_pool.tile([128, 8, 512], FP16, tag="xt",
                                      name=f"xt{ch}")
                    nc.sync.dma_start(
                        out=xt,
                        in_=xT[ch].rearrange("p (d t) -> p d t", d=8))
                    for ct in range(3):
                        ps = ps_all.tile([128, 512], F32, tag="qkvps", bufs=2,
                                         name=f"qkvps{ch}_{ct}")
                        for d in range(8):
                            nc.tensor.matmul(
                                ps,
                                w_sb[:, d, ct * 128:(ct + 1) * 128],
                                xt[:, d, :],
                                start=(d == 0), stop=(d == 7))
                        if ct == 0:
                            nc.vector.tensor_scalar_add(
                                qT_sb[:, ch * 512:(ch + 1) * 512], ps,
                                bias_sb[:, 0:1])
                        elif ct == 1:
                            nc.vector.tensor_scalar_add(
                                kT_sb[:, ch * 512:(ch + 1) * 512], ps,
                                bias_sb[:, 1:2])
                        else:
                            vst = vstage_pool.tile([128, 512], F32, tag="vst",
                                                   name=f"vst{ch}")
                            nc.vector.tensor_scalar_add(vst, ps, bias_sb[:, 2:3])
                            vtp = ps_all.tile([128, 512], F32, tag="qkvps",
                                              bufs=2, name=f"vtps{ch}")
                            for sub in range(4):
                                nc.tensor.transpose(
                                    vtp[:, sub * 128:(sub + 1) * 128],
                                    vst[:, sub * 128:(sub + 1) * 128],
                                    ident)
                            for hh in range(HPC):
                                nc.vector.tensor_copy(
                                    vn_sb[:, b, hh, tt0:tt0 + 4, 0:64],
                                    vtp.rearrange("p (s x) -> p s x", s=4)[
                                        :, :, hh * 64:(hh + 1) * 64])
                        yield

            # ---------------- projection round (emitted as steps) -----------
            def proj_steps(r):
                W = RSL[r]
                c0 = COL0[r]
                # step 0: fetch + normalize
                rhs = proj_pool.tile([128, 8, RS], FP16, tag="rhs",
                                     name=f"rhs{r}")
                nc.gpsimd.dma_start(
                    out=rhs[:, :, 0:W], in_=a2a_out[r][:, 0:128, :].rearrange(
                        "s p x -> p s x"))
                # broadcast the shipped reciprocal rows straight from a2a_out
                rcb = proj_pool.tile([128, 8, RS], FP16, tag="rcb",
                                     name=f"rcb{r}")
                for hh in range(HPC):
                    nc.gpsimd.dma_start(
                        out=rcb[hh * 64:(hh + 1) * 64, :, 0:W],
                        in_=bass.AP(tensor=a2a_out[r].tensor,
                                    offset=(128 + hh) * W,
                                    ap=[[0, 64], [130 * W, 8], [1, W]]))
                # normalize on GPSIMD: its queue is ordered behind the
                # collective anyway, so no other engine's FIFO can head-block
                rhn = proj_pool.tile([128, 8, RS], FP16, tag="rhn",
                                     name=f"rhn{r}")
                nc.gpsimd.tensor_mul(rhn[:, :, 0:W], rhs[:, :, 0:W],
                                     rcb[:, :, 0:W])
                yield
                for od in range(8):
                    pj = ps_all.tile([128, 512], F32, tag="qkvps", bufs=2,
                                     name=f"pj{r}_{od}")
                    for i in range(8):
                        nc.tensor.matmul(
                            pj[:, 0:W],
                            wproj_sb[:, i, od * 128:(od + 1) * 128],
                            rhn[:, i, 0:W],
                            start=(i == 0), stop=(i == 7))
                    ot = out_pool.tile([128, RS], FP16, tag="ot",
                                       name=f"ot{r}_{od}")
                    nc.vector.tensor_scalar_add(ot[:, 0:W], pj[:, 0:W],
                                                bproj_sb[:, od:od + 1])
                    nc.sync.dma_start(
                        out=out[od * 128:(od + 1) * 128, c0:c0 + W],
                        in_=ot[:, 0:W])
                    yield

            # ---------------- attention with filler interleave -------------
            def pump(filler):
                """Emit one filler step; return False when empty."""
                while filler:
                    try:
                        next(filler[0])
                        return True
                    except StopIteration:
                        filler.pop(0)
                return False

            def attention_batch(b, filler, qc_min=None, after_qc=None):
                pumped = 0
                for qc in range(NQC):
                    if qc_min is not None:
                        while pumped < qc_min[qc] and pump(filler):
                            pumped += 1
                    nkt_q = 4 * qc + 4
                    t0 = b * T
                    q0 = qT_sb[0:64, t0 + qc * 512: t0 + (qc + 1) * 512]
                    q1 = qT_sb[64:128, t0 + qc * 512: t0 + (qc + 1) * 512]
                    pv0 = ps_all.tile([65, 512], F32, tag="pv", bufs=2,
                                      name=f"pv0_{b}_{qc}")
                    pv1 = ps_all.tile([65, 512], F32, tag="pv", bufs=2,
                                      name=f"pv1_{b}_{qc}")
                    for kt in range(nkt_q):
                        sc = ps_all.tile([128, 1024], F32, tag="sc", bufs=2,
                                         name=f"sc{b}_{qc}_{kt}")
                        nc.tensor.matmul(
                            sc[:, 0:512],
                            kT_sb[0:64, t0 + kt * 128: t0 + (kt + 1) * 128],
                            q0, start=True, stop=True)
                        nc.tensor.matmul(
                            sc[:, 512:1024],
                            kT_sb[64:128, t0 + kt * 128: t0 + (kt + 1) * 128],
                            q1, start=True, stop=True)
                        ex = exp_pool.tile([128, 1024], FP16, tag="ex",
                                           name=f"ex{b}_{qc}_{kt}")
                        nc.scalar.activation(ex, sc, EXP, scale=0.125,
                                             bias=expb)
                        bj = kt - 4 * qc
                        if bj >= 0:
                            exr = ex.rearrange("p (h x) -> p h x", h=2)
                            nc.vector.tensor_mul(exr, exr, masks[:, bj])
                        # fill the PE gap behind the exp chain
                        if pump(filler):
                            pumped += 1
                        nc.tensor.matmul(
                            pv0, vn_sb[:, b, 0, kt, :], ex[:, 0:512],
                            start=(kt == 0), stop=(kt == nkt_q - 1))
                        nc.tensor.matmul(
                            pv1, vn_sb[:, b, 1, kt, :], ex[:, 512:1024],
                            start=(kt == 0), stop=(kt == nkt_q - 1))
                    # stage unnormalized output + denominators, ship to a2a
                    at = att_pool.tile([128, 512], FP16, tag="at",
                                       name=f"at{b}_{qc}")
                    nc.vector.tensor_copy(at[0:64, :], pv0[0:64, :])
                    nc.vector.tensor_copy(at[64:128, :], pv1[0:64, :])
                    nc.vector.tensor_copy(
                        den_sb[0:1, b, qc * 512:(qc + 1) * 512], pv0[64:65, :])
                    nc.vector.tensor_copy(
                        den_sb[64:65, b, qc * 512:(qc + 1) * 512], pv1[64:65, :])
                    r, d0, nd = rdst(b, qc)
                    nc.sync.dma_start(
                        out=a2a_in[r][d0:d0 + nd, 0:128, :].rearrange(
                            "d p x -> p d x"),
                        in_=at.rearrange("p (d x) -> p d x", d=nd))
                    if after_qc and qc in after_qc:
                        after_qc[qc]()

            def rdst(b, qc):
                if b < 3:
                    return b, 2 * qc, 2
                return 3 + qc // 2, 4 * (qc % 2), 4

            def rc_chain(b, q0, q1):
                """1/denominator for qc in [q0,q1) + ship rc rows to a2a_in."""
                sl = slice(q0 * 512, q1 * 512)
                nc.scalar.activation(lg_sb[:, sl], den_sb[:, b, sl], LOG)
                nc.scalar.activation(den_sb[:, b, sl], lg_sb[:, sl], EXP,
                                     scale=-1.0)
                for qc in range(q0, q1):
                    r, d0, nd = rdst(b, qc)
                    for hh, prow in ((0, 0), (1, 64)):
                        nc.sync.dma_start(
                            out=a2a_in[r][d0:d0 + nd, 128 + hh:129 + hh,
                                          :].rearrange("d h x -> h d x"),
                            in_=den_sb[prow:prow + 1, b,
                                       qc * 512:(qc + 1) * 512].rearrange(
                                           "p (d x) -> p d x", d=nd))

            def a2a(r):
                nc.gpsimd.collective_compute(
                    "AllToAll", mybir.AluOpType.bypass,
                    replica_groups=[list(range(NCORES))],
                    ins=[a2a_in[r][:]], outs=[a2a_out[r][:]])

            def flush(filler):
                while filler:
                    try:
                        next(filler[0])
                    except StopIteration:
                        filler.pop(0)

            # ---------------- schedule ----------------
            # dummy collective: absorb the one-time CC rendezvous/warmup cost
            # during the qkv prologue instead of in front of a2a(0)
            wz = consts.tile([8, 64], FP16)
            nc.vector.memset(wz, 0.0)
            warm_in = dram.tile([NCORES, 1, 8], FP16, name="warmin")
            warm_out = dram.tile([NCORES, 1, 8], FP16, name="warmout")
            nc.gpsimd.dma_start(out=warm_in, in_=wz[:, 0:8])
            nc.gpsimd.collective_compute(
                "AllToAll", mybir.AluOpType.bypass,
                replica_groups=[list(range(NCORES))],
                ins=[warm_in[:]], outs=[warm_out[:]])

            g0 = qkv_steps(0)
            for _ in range(3):             # chunk 0 of batch 0
                next(g0)
            f = [g0, qkv_steps(1)]
            attention_batch(0, f, qc_min=[0, 3, 6, 9])
            flush(f)                       # qkv(1) complete before att(1)
            rc_chain(0, 0, 4)
            a2a(0)
            f = [qkv_steps(2)]
            attention_batch(1, f)
            flush(f)
            rc_chain(1, 0, 4)
            a2a(1)
            f = [qkv_steps(3), proj_steps(0)]
            attention_batch(2, f)
            flush(f)                       # qkv(3) complete before att(3)
            rc_chain(2, 0, 4)
            a2a(2)

            def mid3():
                rc_chain(3, 0, 2)
                a2a(3)
                f.append(proj_steps(3))

            f = [proj_steps(1), proj_steps(2)]
            attention_batch(3, f, after_qc={1: mid3})
            rc_chain(3, 2, 4)
            a2a(4)
            f.append(proj_steps(4))
            flush(f)


def build_nc():
    TOK = B * T
    NCH = TOK // 512
    TS = TOK // NCORES
    nc = bacc.Bacc("TRN2", target_bir_lowering=False, debug=False,
                   enable_asserts=False, num_devices=NCORES)
    io = {
        "xT": nc.dram_tensor("xT", [NCH, 128, 4096], FP16,
                             kind="ExternalInput").ap(),
        "wqkv": nc.dram_tensor("wqkv", [128, 8, 384], FP16,
                               kind="ExternalInput").ap(),
        "bqkv": nc.dram_tensor("bqkv", [128, 3], F32,
                               kind="ExternalInput").ap(),
        "wproj": nc.dram_tensor("wproj", [128, 8, 1024], FP16,
                                kind="ExternalInput").ap(),
        "bproj": nc.dram_tensor("bproj", [128, 8], F32,
                                kind="ExternalInput").ap(),
        "out": nc.dram_tensor("out", [D, TS], FP16,
                              kind="ExternalOutput").ap(),
    }
    with tile.TileContext(nc) as tc:
        emit(tc, io)
    nc.compile()
    return nc


def make_in_maps(x, W_qkv, b_qkv, W_proj, b_proj):
    """Shard host inputs per core."""
    TOK = B * T
    NCH = TOK // 512
    x2 = np.asarray(x, np.float32).reshape(TOK, D)
    # [NCH, 128, 8, 512]: chunk, partition, d-group, token -- contiguous DMA
    xc = np.ascontiguousarray(
        x2.reshape(NCH, 512, 8, 128).transpose(0, 3, 2, 1)
    ).astype(np.float16).reshape(NCH, 128, 4096)
    wproj_f16 = np.ascontiguousarray(
        np.asarray(W_proj, np.float32).reshape(8, 128, D).transpose(1, 0, 2)
    ).astype(np.float16)
    bproj_rs = np.ascontiguousarray(
        np.asarray(b_proj, np.float32).reshape(8, 128).T)  # [128, 8]
    in_maps = []
    for c in range(NCORES):
        cols = []
        bcols = []
        for part in range(3):                            # q, k, v
            for h in (2 * c, 2 * c + 1):
                sl = slice(part * D + h * DH, part * D + (h + 1) * DH)
                cols.append(np.asarray(W_qkv, np.float32)[:, sl])
                bcols.append(np.asarray(b_qkv, np.float32)[sl])
        wq = np.concatenate(cols, axis=1)                # [1024, 384]
        wq = np.ascontiguousarray(
            wq.reshape(8, 128, 384).transpose(1, 0, 2)).astype(np.float16)
        bq = np.ascontiguousarray(
            np.concatenate(bcols).reshape(3, 128).T)     # [128, 3]
        in_maps.append({
            "xT": xc, "wqkv": wq, "bqkv": bq,
            "wproj": wproj_f16, "bproj": bproj_rs,
        })
    return in_maps


def gather_out(results):
    TOK = B * T
    fullT = np.empty((D, TOK), np.float32)
    for c in range(NCORES):
        o = results[c]["out"].astype(np.float32)
        for r in range(3):
            tok0 = r * T + (c // 2) * 512 + (c % 2) * 256
            fullT[:, tok0:tok0 + 256] = o[:, r * 256:(r + 1) * 256]
        for r in (3, 4):
            qc = 2 * (r - 3) + c // 4
            tok0 = 3 * T + qc * 512 + (c % 4) * 128
            col0 = 768 + (r - 3) * 128
            fullT[:, tok0:tok0 + 128] = o[:, col0:col0 + 128]
    return np.ascontiguousarray(fullT.T).reshape(B, T, D)


_NC_CACHE = {}
LAST_EXEC_NS = None


def kernel(x, mask, W_qkv, b_qkv, W_proj, b_proj, trace=False):
    global LAST_EXEC_NS
    key = (B, T)
    if key not in _NC_CACHE:
        _NC_CACHE[key] = build_nc()
    nc = _NC_CACHE[key]
    in_maps = make_in_maps(x, W_qkv, b_qkv, W_proj, b_proj)
    res = run_bass_kernel_spmd(nc, in_maps, core_ids=list(range(NCORES)),
                               trace=trace)
    LAST_EXEC_NS = res.exec_time_ns
    return gather_out(res.results)
